# revision 12
# baseline (speedup 1.0000x reference)
"""SAGAN-style self-attention on 8 trn2 cores: data-parallel over batch.

Per core (one batch image): x^T [256,4096] bf16 in, out^T [256,4096] f32 out.

v2 over the 215.9us baseline:
  - merged Q|K projection (one [64,512] psum + one DVE bias op per tile)
  - T bursts: both units of a pair emitted as 4 back-to-back row-packed
    MMs (strips 0-3) for 4-way PE tile concurrency
  - Z moved out of the per-unit chain into per-tile Z passes (4 quads of
    back-to-back [16,512] DR MMs during the next tile's first uslots)
  - exp split ACT/DVE: ScalarE does cols [0, NT-WD), DVE does the last WD
    cols per unit via the fp8e4m3 bit-trick u8 = round(s*11.54 + 54.75)
    (probe: DVE f32->u8 is exact round-to-nearest; centered rms 2.9% vs
    2.7% for ACT exp + fp8 cast)
  - 1/Z via DRAM round-trip reshape [1,512]->[128,4] so the DVE
    reciprocal runs 128 lanes wide (was 3.3us on 1 lane, now ~0.1us)
  - DMA triggers off ScalarE during attention (exp owns ScalarE)
PSUM: t=2x2 (double-buffered T pairs), o0/o1=2, z=1, f=1 -> 8.
"""

import sys

if "/opt/trn_rl_repo" not in sys.path:
    sys.path.insert(0, "/opt/trn_rl_repo")

import ml_dtypes
import numpy as np

import concourse.bass as bass
import concourse.mybir as mybir
import concourse.tile as tile
from concourse.bass_utils import run_bass_kernel_spmd

B, H, W, C = 8, 64, 64, 256
KEY = 32
N = H * W          # 4096 tokens
NT = 512           # query tile (free dim per matmul)
NTILES = N // NT   # 8
MB = 128           # key block
NMB = N // MB      # 32
NU = NMB // 2      # 16 units (pairs of key blocks) per tile

DVE_UNITS = {2, 5, 8, 11, 13, 15}   # units whose exp runs on DVE (fast-exp)
EXPA = 8.0 / float(np.log(2.0))
EXPB = 55.55       # zero-mean vs true exp over the score distribution

BF16 = mybir.dt.bfloat16
F32 = mybir.dt.float32
F8 = mybir.dt.float8e4
U8 = mybir.dt.uint8
FT = mybir.ActivationFunctionType
DR = mybir.MatmulPerfMode.DoubleRow
ADD = mybir.AluOpType.add
MUL = mybir.AluOpType.mult


def build_nc() -> bass.Bass:
    nc = bass.Bass()

    xT = nc.declare_dram_parameter("xT", [2, 128, N], BF16, isOutput=False)
    wfg = nc.declare_dram_parameter("wfg", [128, 2, 64], BF16, isOutput=False)
    wh = nc.declare_dram_parameter("wh", [2, 128, C], BF16, isOutput=False)
    wo = nc.declare_dram_parameter("wo", [2, 128, C], BF16, isOutput=False)
    bfgT = nc.declare_dram_parameter("bfgT", [64, 1], F32, isOutput=False)
    bhp = nc.declare_dram_parameter("bhp", [1, C], BF16, isOutput=False)
    bop = nc.declare_dram_parameter("bop", [1, C], BF16, isOutput=False)
    outT = nc.declare_dram_parameter("outT", [2, 128, N], F32, isOutput=True)

    # per-tile DRAM scratch for the 1/Z reshape round-trips
    zdram = [nc.dram_tensor(f"zdram{i}", [1, NT], BF16, kind="Internal")
             for i in range(NTILES)]
    rdram = [nc.dram_tensor(f"rdram{i}", [128, NT // 128], BF16, kind="Internal")
             for i in range(NTILES)]

    with tile.TileContext(nc) as tc:
        with (
            tc.tile_pool(name="const", bufs=1) as const,
            tc.tile_pool(name="xp", bufs=1) as xp,
            tc.tile_pool(name="vp", bufs=1) as vp,
            tc.tile_pool(name="qk", bufs=1) as qk,
            tc.tile_pool(name="ep", bufs=1) as ep,
            tc.tile_pool(name="osb", bufs=2) as osbp,
            tc.tile_pool(name="frp", bufs=2) as frp,
            tc.tile_pool(name="zsp", bufs=2) as zsp,
            tc.tile_pool(name="outp", bufs=3) as outp,
            tc.tile_pool(name="pt", bufs=2, space="PSUM") as pt,
            tc.tile_pool(name="po", bufs=1, space="PSUM") as po,
            tc.tile_pool(name="pz", bufs=1, space="PSUM") as pz,
            tc.tile_pool(name="pf", bufs=1, space="PSUM") as pf,
        ):
            # ---- constants ----
            ones2 = const.tile([128, 2, 16], F8)    # Z DoubleRow lhsT
            ones_m = const.tile([1, 128], BF16)     # K=1 broadcast lhsT
            nc.vector.memset(ones2, 1.0)
            nc.vector.memset(ones_m, 1.0)

            wfg_sb = const.tile([128, 2, 64], BF16)
            wh_sb = const.tile([128, 2, C], BF16)
            wo_sb = const.tile([128, 2, C], BF16)
            bfg_sb = const.tile([64, 1], F32)
            bh_sb = const.tile([1, C], BF16)
            bo_sb = const.tile([1, C], BF16)
            bh_bc = const.tile([128, C], BF16)

            # DMA trigger assignment: scalar only in the pure prologue
            # (before the first exp lands on ACT); sync/gpsimd afterwards.
            rt_i = 0

            def dma_rt(out, in_):
                nonlocal rt_i
                eng = (nc.sync, nc.gpsimd)[rt_i % 2]
                rt_i += 1
                eng.dma_start(out=out, in_=in_)

            # x chunks in 1024-col tiles (fewer triggers than 512)
            xts = [
                [xp.tile([128, 2 * NT], BF16, name=f"xt{cc}_{h}") for h in range(4)]
                for cc in range(2)
            ]

            def xdma(h2, eng=None):
                for cc in range(2):
                    e = eng if eng is not None else (nc.sync, nc.gpsimd)[cc]
                    e.dma_start(
                        out=xts[cc][h2], in_=xT[cc, :, h2 * 2 * NT:(h2 + 1) * 2 * NT])

            # prologue DMAs: x0, weights on scalar; x1.. interleaved
            xdma(0, nc.scalar)
            nc.scalar.dma_start(out=wfg_sb, in_=wfg[:])
            nc.scalar.dma_start(out=bfg_sb, in_=bfgT[:])
            xdma(1, nc.scalar)
            for cc in range(2):
                nc.scalar.dma_start(out=wh_sb[:, cc, :], in_=wh[cc])
            nc.scalar.dma_start(out=bh_sb, in_=bhp[:])
            for cc in range(2):
                nc.scalar.dma_start(out=wo_sb[:, cc, :], in_=wo[cc])
            nc.scalar.dma_start(out=bo_sb, in_=bop[:])
            xdma(2)
            xdma(3)

            def xs(cc, start, width):
                h2 = start // (2 * NT)
                assert (start + width - 1) // (2 * NT) == h2
                o = start - h2 * 2 * NT
                return xts[cc][h2][:, o:o + width]

            pp_i = 0

            def proj_psum(shape):
                # rotate pz/pf for projection psums (pt stays clean for T)
                nonlocal pp_i
                pp_i += 1
                if pp_i % 2:
                    return pz.tile(shape, F32, tag="z", name=f"projps{pp_i}")
                return pf.tile(shape, F32, tag="f", name=f"projps{pp_i}")

            bh_ps = proj_psum([128, C])
            nc.tensor.matmul(bh_ps, ones_m, bh_sb, start=True, stop=True)
            nc.vector.tensor_copy(out=bh_bc, in_=bh_ps)

            # ---- merged Q|K projection per tile: rows 0:32 = Q^T, 32:64 = K^T
            qkt = [qk.tile([64, NT], BF16, name=f"qkt{nt}") for nt in range(NTILES)]
            qts = [qk.tile([128, NT], BF16, name=f"qts{nt}") for nt in range(NTILES)]
            kt_stack = [qk.tile([128, MB], BF16, name=f"kstk{g}") for g in range(NTILES)]

            def qkproj(nt):
                ps = proj_psum([64, NT])
                for cc in range(2):
                    nc.tensor.matmul(
                        ps, wfg_sb[:, cc, :], xs(cc, nt * NT, NT),
                        start=(cc == 0), stop=(cc == 1),
                    )
                nc.vector.tensor_scalar_add(qkt[nt], ps, bfg_sb)
                # replicate Q rows into strips 1-3 (strip 0 reads qkt directly)
                for i in range(1, 4):
                    nc.gpsimd.dma_start(
                        out=qts[nt][32 * i:32 * (i + 1), :], in_=qkt[nt][0:32, :])
                # K row strips for T lhsT
                for i in range(4):
                    nc.sync.dma_start(
                        out=kt_stack[nt][32 * i:32 * (i + 1), :],
                        in_=qkt[nt][32:64, i * MB:(i + 1) * MB])

            def qstrip(nt, s):
                if s == 0:
                    return qkt[nt][0:32, :]
                return qts[nt][32 * s:32 * (s + 1), :]

            # ---- V projection -> fp8 pair tiles [128, 2, C] ----------------
            v2 = [
                vp.tile([128, 2, C], F8, tag=f"v{p}", name=f"v{p}")
                for p in range(NU)
            ]

            def vpair(pair):
                for mem in range(2):
                    mb = 2 * pair + mem
                    ps = proj_psum([128, C])
                    for cc in range(2):
                        nc.tensor.matmul(
                            ps, xs(cc, mb * MB, MB), wh_sb[:, cc, :],
                            start=(cc == 0), stop=(cc == 1),
                        )
                    nc.vector.tensor_tensor(
                        out=v2[pair][:, mem, :], in0=ps, in1=bh_bc, op=ADD,
                    )

            # ---- attention state ------------------------------------------
            # e tiles: uint8 bit-pattern of fp8e4m3 exp values; 2 tiles live
            e_t = [
                [ep.tile([128, 2, NT], U8, name=f"e{par}_{u}") for u in range(NU)]
                for par in range(2)
            ]
            state = {}

            def emit_T2(nt, b):
                """T burst: units (2b, 2b+1), 4 MMs on strips 0..3."""
                for du in range(2):
                    u = 2 * b + du
                    t_ps = pt.tile([128, 2, NT], F32, tag="t", name=f"t{nt}_{u}")
                    for j in range(2):
                        s = (2 * u + j) % 4
                        nc.tensor.matmul(
                            t_ps[:, j, :],
                            kt_stack[b][32 * s:32 * (s + 1), :],
                            qstrip(nt, s),
                            start=True, stop=True,
                            tile_position=(32 * s, 0),
                        )
                    state[(nt, u)] = t_ps

            def emit_exp(nt, u):
                """Whole-unit exp on ONE engine (single writer per e tile,
                single reader per t psum -> fewer sync waits). DVE units use
                the fp8 bit-trick; zero-mean bias so the DVE-keys vs ACT-keys
                softmax weighting stays untilted."""
                t_ps = state.pop((nt, u))
                e = e_t[nt % 2][u]
                if u in DVE_UNITS:
                    nc.vector.tensor_scalar(
                        out=e[:], in0=t_ps[:],
                        scalar1=EXPA, scalar2=EXPB, op0=MUL, op1=ADD,
                    )
                else:
                    nc.scalar.activation(
                        out=e[:].bitcast(F8), in_=t_ps[:], func=FT.Exp,
                    )

            def emit_o(nt, u):
                if u == 0:
                    state[("o", nt)] = [
                        po.tile([128, NT], F32, tag="o0", name=f"o0_{nt}"),
                        po.tile([128, NT], F32, tag="o1", name=f"o1_{nt}"),
                    ]
                o = state[("o", nt)]
                first, last = u == 0, u == NU - 1
                for cc in range(2):
                    nc.tensor.matmul(
                        o[cc],
                        v2[u][:, :, cc * 128:(cc + 1) * 128],
                        e_t[nt % 2][u][:].bitcast(F8),
                        start=first, stop=last,
                        perf_mode=DR,
                    )
                if last:
                    tail_osb(nt)

            def z_quad(nt, q):
                """4 back-to-back Z MMs (units 4q..4q+3) into zz[nt%...]."""
                if q == 0:
                    state[("zz", nt)] = pz.tile([16, NT], F32, tag="z", name=f"zz{nt}")
                zz = state[("zz", nt)]
                for du in range(4):
                    u = 4 * q + du
                    nc.tensor.matmul(
                        zz, ones2, e_t[nt % 2][u][:].bitcast(F8),
                        start=(u == 0), stop=(u == NU - 1),
                        perf_mode=DR,
                    )

            def tail_osb(nt):
                o = state[("o", nt)]
                ot0 = osbp.tile([128, NT], BF16, tag="os0", name=f"os0_{nt}")
                nc.vector.tensor_copy(out=ot0, in_=o[0])
                ot1 = osbp.tile([128, NT], BF16, tag="os1", name=f"os1_{nt}")
                nc.vector.tensor_copy(out=ot1, in_=o[1])
                state[("osb", nt)] = (ot0, ot1)

            def tail_z(nt):
                """After zz(nt) stops: bf16 copy for the bias MM, and kick the
                1/Z DRAM round-trip (all 4 DMAs on sync: queue-ordered)."""
                zz = state.pop(("zz", nt))
                zbf = zsp.tile([1, NT], BF16, tag="zbf", name=f"zbf{nt}")
                nc.vector.tensor_copy(out=zbf, in_=zz[0:1, :])
                state[("zbf", nt)] = zbf
                nc.sync.dma_start(out=zdram[nt][:], in_=zbf)
                zs = zsp.tile([128, NT // 128], BF16, tag="zs", name=f"zs{nt}")
                nc.sync.dma_start(
                    out=zs, in_=zdram[nt][:].rearrange("i (p f) -> (i p) f", p=128))
                zr = zsp.tile([128, NT // 128], F32, tag="zr", name=f"zr{nt}")
                nc.vector.reciprocal(out=zr, in_=zs)
                zrb = zsp.tile([128, NT // 128], BF16, tag="zrb", name=f"zrb{nt}")
                nc.vector.tensor_copy(out=zrb, in_=zr)
                nc.sync.dma_start(out=rdram[nt][:], in_=zrb)
                zrbp = zsp.tile([1, NT], BF16, tag="zrbp", name=f"zrbp{nt}")
                nc.sync.dma_start(
                    out=zrbp, in_=rdram[nt][:].rearrange("p f -> () (p f)"))
                state[("zrb", nt)] = zrbp

            def tail_f(nt, cp):
                """f = bo (x) Z + Wo^T @ osb for half cp (normalize after);
                immediate bf16 evacuation so the f bank frees fast."""
                osb = state[("osb", nt)]
                csl = slice(cp * 128, (cp + 1) * 128)
                f_ps = pf.tile([128, NT], F32, tag="f", name=f"f{cp}_{nt}")
                nc.tensor.matmul(
                    f_ps, bo_sb[:, csl], state[("zbf", nt)], start=True, stop=False,
                )
                for cc in range(2):
                    nc.tensor.matmul(
                        f_ps, wo_sb[:, cc, csl], osb[cc],
                        start=False, stop=(cc == 1),
                    )
                fr = frp.tile([128, NT], BF16, tag=f"fr{cp}", name=f"fr{cp}_{nt}")
                nc.vector.tensor_copy(out=fr, in_=f_ps)
                state[(f"f{cp}", nt)] = fr

            def tail_zb(nt):
                """1/Z broadcast to 128 partitions (borrows the f slot)."""
                zb_ps = pf.tile([128, NT], F32, tag="f", name=f"zbp{nt}")
                nc.tensor.matmul(
                    zb_ps, ones_m, state.pop(("zrb", nt)), start=True, stop=True)
                zb = zsp.tile([128, NT], BF16, tag="zb", name=f"zb{nt}")
                nc.vector.tensor_copy(out=zb, in_=zb_ps)
                state[("zb", nt)] = zb

            def tail_out(nt, cp):
                nsl = slice(nt * NT, (nt + 1) * NT)
                out_sb = outp.tile([128, NT], F32, tag="out", name=f"out{cp}_{nt}")
                nc.vector.tensor_tensor(
                    out=out_sb, in0=state.pop((f"f{cp}", nt)),
                    in1=state[("zb", nt)], op=MUL,
                )
                dma_rt(outT[cp, :, nsl], out_sb)

            # ---- schedule --------------------------------------------------
            pending = []   # (nt, u) exp'd, O' not yet emitted
            LAG = 3

            def drain_o(target):
                while len(pending) > target:
                    emit_o(*pending.pop(0))

            # prologue: projections for tiles 0-1, V pairs 0-1
            qkproj(0)
            qkproj(1)
            vpair(0)
            vpair(1)

            for nt in range(NTILES):
                for b in range(8):
                    emit_T2(nt, b)
                    # interleaved work while this tile's T/exp pipeline runs
                    if nt == 0:
                        if b < 6:
                            qkproj(b + 2)
                        if b < 7:
                            vpair(2 * b + 2)
                            vpair(2 * b + 3)
                    else:
                        if b < 4:
                            z_quad(nt - 1, b)
                        elif b == 4:
                            tail_z(nt - 1)
                        elif b == 5:
                            tail_f(nt - 1, 0)
                        elif b == 6:
                            tail_f(nt - 1, 1)
                        if b == 1 and nt > 1:
                            tail_zb(nt - 2)
                        if b == 2 and nt > 1:
                            tail_out(nt - 2, 0)
                            tail_out(nt - 2, 1)
                    emit_exp(nt, 2 * b)
                    emit_exp(nt, 2 * b + 1)
                    pending.append((nt, 2 * b))
                    pending.append((nt, 2 * b + 1))
                    drain_o(LAG)

            drain_o(0)
            # rampdown: kick tile 7's z-chain first so its DMA latency
            # overlaps tile 6's tail end
            last = NTILES - 1
            for q in range(4):
                z_quad(last, q)
            tail_z(last)
            tail_zb(last - 1)
            tail_out(last - 1, 0)
            tail_out(last - 1, 1)
            tail_f(last, 0)
            tail_f(last, 1)
            tail_zb(last)
            tail_out(last, 0)
            tail_out(last, 1)

    _split_multiwaits(nc)
    return nc


def _split_multiwaits(nc: bass.Bass) -> None:
    """This container's walrus accepts at most ONE sync-wait per instruction
    (CoreV3GenImpl setupSyncWait). Tile emits multi-wait instructions; split
    the excess waits onto EventSemaphore carriers inserted just before the
    instruction on the same engine."""
    import json as _json

    data = _json.loads(mybir.module_to_json_bytes(nc.m))
    uid = 0
    for fn in data["functions"]:
        for bb in fn["blocks"]:
            new = []
            for inst in bb["instructions"]:
                si = inst.get("sync_info")
                waits = (si or {}).get("on_wait") or []
                if len(waits) > 1:
                    for wcmd in waits[:-1]:
                        uid += 1
                        new.append({
                            "debug": inst.get("debug", 0),
                            "engine": inst["engine"],
                            "ins": [], "outs": [],
                            "name": f"syncw-{uid}",
                            "opcode": "EventSemaphore",
                            "sync_info": {"on_update": [], "on_wait": [wcmd]},
                        })
                    si["on_wait"] = [waits[-1]]
                new.append(inst)
            bb["instructions"] = new
    nc.m = mybir.module_from_json_bytes(_json.dumps(data).encode())


_NC = None


def _get_nc():
    global _NC
    if _NC is None:
        _NC = build_nc()
    return _NC


def _prep_maps(x, Wf, bf, Wg, bg, Wh, bh, Wo, bo):
    bft = ml_dtypes.bfloat16
    wfg = np.concatenate([Wf, Wg], axis=1)          # [256, 64]
    shared = {
        "wfg": np.ascontiguousarray(
            wfg.reshape(2, 128, 64).transpose(1, 0, 2).astype(bft)),
        "wh": np.ascontiguousarray(Wh.reshape(2, 128, C).astype(bft)),
        "wo": np.ascontiguousarray(Wo.reshape(2, 128, C).astype(bft)),
        "bfgT": np.ascontiguousarray(
            np.concatenate([bf, bg]).reshape(64, 1).astype(np.float32)),
        "bhp": np.ascontiguousarray(bh.reshape(1, C).astype(bft)),
        "bop": np.ascontiguousarray(bo.reshape(1, C).astype(bft)),
    }
    in_maps = []
    for b in range(B):
        xTb = np.ascontiguousarray(
            x[b].reshape(N, C).T.astype(bft).reshape(2, 128, N)
        )
        m = dict(shared)
        m["xT"] = xTb
        in_maps.append(m)
    return in_maps


def run(x, Wf, bf, Wg, bg, Wh, bh, Wo, bo, trace=False, **kw):
    x = np.asarray(x, dtype=np.float32)
    in_maps = _prep_maps(
        x, *(np.asarray(a, dtype=np.float32) for a in (Wf, bf, Wg, bg, Wh, bh, Wo, bo))
    )
    res = run_bass_kernel_spmd(_get_nc(), in_maps, list(range(B)), trace=trace, **kw)
    out = np.empty((B, H, W, C), dtype=np.float32)
    for b in range(B):
        oT = np.asarray(res.results[b]["outT"], dtype=np.float32).reshape(C, N)
        out[b] = oT.T.reshape(H, W, C)
    return out, res


def kernel(x, Wf, bf, Wg, bg, Wh, bh, Wo, bo):
    out, _ = run(x, Wf, bf, Wg, bg, Wh, bh, Wo, bo)
    return out


# revision 23
# speedup vs baseline: 1.1834x; 1.1834x over previous
"""SAGAN-style self-attention on 8 trn2 cores: data-parallel over batch.

Per core (one batch image): x^T [256,4096] bf16 in, out^T [256,4096] f32 out.
Projections (Q/K/V) chase the x DMAs (round-robin across SP/ACT/Pool
triggers); tile 0's attention units interleave with the tail of that chase.

  QT/KT = W^T @ xT + b        [32, 4096]  (bias via DVE tensor_scalar; K goes
                              straight into per-group kt_stack row strips)
  V     = x @ Wh + bh         fp8e4 pair tiles [128, 2, 256] ([keys, pair, c])
  per 512-query tile, per unit = pair of 128-key blocks (16 units/tile):
    T    = KT_strips.T @ QT    [128 keys, 2*512] 2-way row-packed (K=32),
                               t psum double-buffered so exp(u) || T(u+1)
    expT = exp(T)              ScalarE, PSUM->SBUF, fp8e4 (|s|<~5, no max-sub)
    O'  += V2_pair.T @ E2_pair [256, 512] fp8 DoubleRow PSUM accum
    Z   += ones2.T @ E2_pair   [16, 512] fp8 DoubleRow (rows identical)
  tail (deferred, off the PE critical path; O'/Z of the next tile are held
  back a few units so PSUM-evacuation WARs are covered by T-pack work):
    osb  = O' -> bf16 (DVE, frees o banks early); zf32 = Z -> SBUF (frees zz)
    zr   = 1/Z (DVE reciprocal, reads the SBUF copy)
    f    = bo (x) Z (K=1 preload) + Wo^T @ osb   (normalize AFTER projection:
    fraw = f -> bf16 (DVE, frees the f bank)      the division commutes)
    zb   = ones (x) (1/Z) broadcast [128, 512] (PE, borrows the f slot)
    out  = fraw * zb (DVE) -> DMA
PSUM banks: t=2x2 (double-buffered T), o0/o1=2, zz=1, f=1 -> 8 exactly.
fp8 E/V numerics: rel-L2 vs f64 reference = 1.47e-2 (gate 2e-2).
"""

import sys

if "/opt/trn_rl_repo" not in sys.path:
    sys.path.insert(0, "/opt/trn_rl_repo")

import ml_dtypes
import numpy as np

import concourse.bass as bass
import concourse.mybir as mybir
import concourse.tile as tile
from concourse.bass_utils import run_bass_kernel_spmd

B, H, W, C = 8, 64, 64, 256
KEY = 32
N = H * W          # 4096 tokens
NT = 512           # query tile (free dim per matmul)
NTILES = N // NT   # 8
MB = 128           # key block
NMB = N // MB      # 32
GRP = 4            # key blocks per group (one per PE row strip)
NGRP = NMB // GRP  # 8

BF16 = mybir.dt.bfloat16
F32 = mybir.dt.float32
F8 = mybir.dt.float8e4
U8 = mybir.dt.uint8
FT = mybir.ActivationFunctionType
DR = mybir.MatmulPerfMode.DoubleRow

WD = 128           # exp cols per unit half offloaded to DVE fast-exp
EXPA = 8.0 / float(np.log(2.0))
EXPB = 55.55       # zero-mean fp8e4m3 bit-trick bias (probe: exact u8 RN)


def build_nc() -> bass.Bass:
    nc = bass.Bass()

    xT = nc.declare_dram_parameter("xT", [2, 128, N], BF16, isOutput=False)
    wf = nc.declare_dram_parameter("wf", [2, 128, KEY], BF16, isOutput=False)
    wg = nc.declare_dram_parameter("wg", [2, 128, KEY], BF16, isOutput=False)
    wh = nc.declare_dram_parameter("wh", [2, 128, C], BF16, isOutput=False)
    wo = nc.declare_dram_parameter("wo", [2, 128, C], BF16, isOutput=False)
    bfT = nc.declare_dram_parameter("bfT", [KEY, 1], F32, isOutput=False)
    bgT = nc.declare_dram_parameter("bgT", [KEY, 1], F32, isOutput=False)
    bhp = nc.declare_dram_parameter("bhp", [1, C], BF16, isOutput=False)
    bop = nc.declare_dram_parameter("bop", [1, C], BF16, isOutput=False)
    outT = nc.declare_dram_parameter("outT", [2, 128, N], F32, isOutput=True)

    # per-tile DRAM scratch for the 1/Z reshape round-trip: [1,512] on one
    # partition -> [128,4] so the DVE reciprocal runs 128 lanes wide
    zdram = [nc.dram_tensor(f"zdram{i}", [1, NT], BF16, kind="Internal")
             for i in range(NTILES)]
    rdram = [nc.dram_tensor(f"rdram{i}", [128, NT // 128], BF16, kind="Internal")
             for i in range(NTILES)]

    with tile.TileContext(nc) as tc:
        with (
            tc.tile_pool(name="const", bufs=1) as const,
            tc.tile_pool(name="xp", bufs=1) as xp,
            tc.tile_pool(name="vp", bufs=1) as vp,
            tc.tile_pool(name="qk", bufs=1) as qk,
            tc.tile_pool(name="ep", bufs=5) as ep,
            tc.tile_pool(name="osb", bufs=2) as osbp,
            tc.tile_pool(name="frp", bufs=2) as frp,
            tc.tile_pool(name="zsp", bufs=2) as zsp,
            tc.tile_pool(name="outp", bufs=3) as outp,
            tc.tile_pool(name="pt", bufs=2, space="PSUM") as pt,
            tc.tile_pool(name="po", bufs=1, space="PSUM") as po,
            tc.tile_pool(name="pz", bufs=1, space="PSUM") as pz,
            tc.tile_pool(name="pf", bufs=1, space="PSUM") as pf,
        ):
            # ---- constants ----
            ones2 = const.tile([128, 2, 16], F8)    # Z DoubleRow lhsT
            # (16-wide: DR weights need pair-step %16B == 0; rows identical)
            ones_m = const.tile([1, 128], BF16)     # K=1 broadcast lhsT
            nc.vector.memset(ones2, 1.0)
            nc.vector.memset(ones_m, 1.0)

            wf_sb = const.tile([128, 2, KEY], BF16)
            wg_sb = const.tile([128, 2, KEY], BF16)
            wh_sb = const.tile([128, 2, C], BF16)
            wo_sb = const.tile([128, 2, C], BF16)
            bf_sb = const.tile([KEY, 1], F32)
            bg_sb = const.tile([KEY, 1], F32)
            bh_sb = const.tile([1, C], BF16)
            bo_sb = const.tile([1, C], BF16)
            # DMA trigger round-robin across the three HWDGE/SWDGE engines:
            # each trigger costs ~600ns of that engine's sequencer, and a
            # single queue would serialize the whole input load (~40us).
            dma_i = 0
            dma_engs = [nc.sync, nc.scalar, nc.gpsimd]

            def dma_rr(out, in_):
                nonlocal dma_i
                eng = dma_engs[dma_i % len(dma_engs)]
                dma_i += 1
                eng.dma_start(out=out, in_=in_)
            # bh broadcast to all 128 partitions (for V bias add on DVE),
            # via K=1 ones matmul into a borrowed PSUM slot
            bh_bc = const.tile([128, C], BF16)

            # xT chunks in 512-col tiles (projections start on first slice)
            xts = [
                [xp.tile([128, NT], BF16, name=f"xt{cc}_{h}") for h in range(NTILES)]
                for cc in range(2)
            ]
            # x chunks first (K proj group g needs x tile h=g), weights
            # interleaved right behind the first x pair
            def xdma(h):
                for cc in range(2):
                    dma_rr(xts[cc][h], xT[cc, :, h * NT:(h + 1) * NT])

            xdma(0)
            for cc in range(2):
                dma_rr(wg_sb[:, cc, :], wg[cc])
                dma_rr(wf_sb[:, cc, :], wf[cc])
            dma_rr(bg_sb, bgT[:])
            dma_rr(bf_sb, bfT[:])
            xdma(1)
            for cc in range(2):
                dma_rr(wh_sb[:, cc, :], wh[cc])
            dma_rr(bh_sb, bhp[:])
            xdma(2)
            for cc in range(2):
                dma_rr(wo_sb[:, cc, :], wo[cc])
            dma_rr(bo_sb, bop[:])
            for h in range(3, NTILES):
                xdma(h)

            def xs(cc, start, width):
                h = start // NT
                assert (start + width - 1) // NT == h
                return xts[cc][h][:, start - h * NT: start - h * NT + width]

            pp_i = 0

            def proj_psum(shape):
                # borrow the "t"/"f" slots (alternating) before attention
                nonlocal pp_i
                pp_i += 1
                if pp_i % 2:
                    return pt.tile(shape, F32, tag="t", name=f"projps{pp_i}")
                return pf.tile(shape, F32, tag="f", name=f"projps{pp_i}")

            bh_ps = proj_psum([128, C])
            nc.tensor.matmul(bh_ps, ones_m, bh_sb, start=True, stop=True)
            nc.vector.tensor_copy(out=bh_bc, in_=bh_ps)

            # ---- K projection; per-group kt_stack tiles (so T of group g
            # only waits for group g), regrouped into row strips by
            # partition-shifting SBUF->SBUF DMAs
            kt_stack = [
                qk.tile([128, MB], BF16, name=f"kstk{g}") for g in range(NGRP)
            ]

            def kproj(g):
                ps = proj_psum([KEY, NT])
                for cc in range(2):
                    nc.tensor.matmul(
                        ps, wg_sb[:, cc, :], xs(cc, g * NT, NT),
                        start=(cc == 0), stop=(cc == 1),
                    )
                ktg = qk.tile([KEY, NT], BF16, name=f"kts{g}")
                nc.vector.tensor_scalar_add(ktg, ps, bg_sb)
                for i in range(4):
                    dma_rr(
                        kt_stack[g][32 * i:32 * (i + 1), :],
                        ktg[:, i * MB:(i + 1) * MB],
                    )

            # ---- per-query-tile Q tiles (replicated into 4 row strips) -----
            qts = [
                qk.tile([128, NT], BF16, name=f"qts{nt}") for nt in range(NTILES)
            ]

            def f_psum(shape):
                nonlocal pp_i
                pp_i += 1
                return pf.tile(shape, F32, tag="f", name=f"fps{pp_i}")

            def qproj(nt, psum_fn=None):
                ps = (psum_fn or proj_psum)([KEY, NT])
                for cc in range(2):
                    nc.tensor.matmul(
                        ps, wf_sb[:, cc, :], xs(cc, nt * NT, NT),
                        start=(cc == 0), stop=(cc == 1),
                    )
                nc.vector.tensor_scalar_add(qts[nt][0:KEY, :], ps, bf_sb)
                for i in range(1, 4):
                    nc.sync.dma_start(
                        out=qts[nt][32 * i:32 * (i + 1), :], in_=qts[nt][0:KEY, :]
                    )

            # ---- V projection -> fp8 pair tiles [128, 2, C] ----------------
            v2 = [
                vp.tile([128, 2, C], F8, tag=f"v{p}", name=f"v{p}")
                for p in range(NMB // 2)
            ]

            def vpair(pair, psum_fn):
                for mem in range(2):
                    mb = 2 * pair + mem
                    ps = psum_fn()
                    for cc in range(2):
                        nc.tensor.matmul(
                            ps, xs(cc, mb * MB, MB), wh_sb[:, cc, :],
                            start=(cc == 0), stop=(cc == 1),
                        )
                    nc.vector.tensor_tensor(
                        out=v2[pair][:, mem, :], in0=ps, in1=bh_bc,
                        op=mybir.AluOpType.add,
                    )

            # prologue: Q for tiles 0/1 and the first two K groups / four V
            # pairs; K groups 2..7 (+ V pairs) interleave with tile 0's units
            qproj(0)
            qproj(1)
            for g in range(2):
                kproj(g)
                vpair(2 * g, lambda: proj_psum([128, C]))
                vpair(2 * g + 1, lambda: proj_psum([128, C]))

            # ---- attention: pipelined over (query-tile, pair-unit) ----------
            # unit u = one pair of key blocks (2u, 2u+1); 16 units per tile.
            # T psum is [128, 2*NT] (2 banks) double-buffered so exp(u) and
            # T(u+1) overlap; row strips alternate (0,1)/(2,3) across units.
            NU = NMB // 2  # 16
            state = {}  # nt -> dict with live tiles for the tail

            def emit_oz(nt, u, e_sb):
                if u == 0:
                    state[nt] = {
                        "o": [po.tile([128, NT], F32, tag="o0", name=f"o0_{nt}"),
                              po.tile([128, NT], F32, tag="o1", name=f"o1_{nt}")],
                        "zz": pz.tile([16, NT], F32, tag="z", name=f"z{nt}"),
                    }
                st = state[nt]
                first, last = u == 0, u == NU - 1
                for cc in range(2):
                    nc.tensor.matmul(
                        st["o"][cc],
                        v2[u][:, :, cc * 128:(cc + 1) * 128],
                        e_sb,
                        start=first, stop=last,
                        perf_mode=DR,
                    )
                nc.tensor.matmul(
                    st["zz"], ones2, e_sb,
                    start=first, stop=last,
                    perf_mode=DR,
                )

            def tail1(nt):
                """PSUM evacuation; emit BEFORE next tile's first O'/Z.
                zz is freed by one fast copy (the slow reciprocal reads
                the SBUF copy later)."""
                st = state[nt]
                ot0 = osbp.tile([128, NT], BF16, tag="os0", name=f"os0_{nt}")
                nc.vector.tensor_copy(out=ot0, in_=st["o"][0])
                ot1 = osbp.tile([128, NT], BF16, tag="os1", name=f"os1_{nt}")
                nc.vector.tensor_copy(out=ot1, in_=st["o"][1])
                st["osb0"], st["osb1"] = ot0, ot1
                zbf = zsp.tile([1, NT], BF16, tag="zbf", name=f"zbf{nt}")
                nc.vector.tensor_copy(out=zbf, in_=st["zz"][0:1, :])
                st["zbf"] = zbf
                # 1/Z via DRAM round-trip reshape: the [1,512] vector lives on
                # one partition (DVE reciprocal would run on 1 lane, 8 cyc/elem
                # = 3.3us); bounce it through DRAM as [128,4] instead. All 4
                # DMAs on the sync queue so they execute in order; the chain
                # has ~8 units of slack before tail2z consumes zrb.
                nc.sync.dma_start(out=zdram[nt][:], in_=zbf)
                zs = zsp.tile([128, NT // 128], BF16, tag="zs", name=f"zs{nt}")
                nc.sync.dma_start(
                    out=zs, in_=zdram[nt][:].rearrange("i (p f) -> (i p) f", p=128))
                zr = zsp.tile([128, NT // 128], F32, tag="zr", name=f"zr{nt}")
                nc.vector.reciprocal(out=zr, in_=zs)
                zrb = zsp.tile([128, NT // 128], BF16, tag="zrb", name=f"zrb{nt}")
                nc.vector.tensor_copy(out=zrb, in_=zr)
                nc.sync.dma_start(out=rdram[nt][:], in_=zrb)
                zrbp = zsp.tile([1, NT], BF16, tag="zrbp", name=f"zrbp{nt}")
                nc.sync.dma_start(
                    out=zrbp, in_=rdram[nt][:].rearrange("p f -> () (p f)"))
                st["zrb"] = zrbp

            def tail2(nt, cp):
                """out-proj half cp: f = bo (x) Z + Wo^T @ osb; fraw; defer mul."""
                st = state[nt]
                csl = slice(cp * 128, (cp + 1) * 128)
                f_ps = pf.tile([128, NT], F32, tag="f", name=f"f{cp}_{nt}")
                nc.tensor.matmul(
                    f_ps, bo_sb[:, csl], st["zbf"], start=True, stop=False,
                )
                for cc in range(2):
                    nc.tensor.matmul(
                        f_ps, wo_sb[:, cc, csl], st[f"osb{cc}"],
                        start=False, stop=(cc == 1),
                    )
                fr = frp.tile([128, NT], BF16, tag=f"fr{cp}", name=f"fr{cp}_{nt}")
                nc.vector.tensor_copy(out=fr, in_=f_ps)
                st[f"fr{cp}"] = fr

            def tail2z(nt):
                """1/Z broadcast to 128 partitions (borrows the f slot);
                deferred past the reciprocal's latency."""
                st = state[nt]
                zb_ps = pf.tile([128, NT], F32, tag="f", name=f"zbp{nt}")
                nc.tensor.matmul(zb_ps, ones_m, st["zrb"], start=True, stop=True)
                zb = zsp.tile([128, NT], BF16, tag="zb", name=f"zb{nt}")
                nc.vector.tensor_copy(out=zb, in_=zb_ps)
                st["zb"] = zb

            def tail3(nt, cp):
                st = state[nt]
                nsl = slice(nt * NT, (nt + 1) * NT)
                out_sb = outp.tile([128, NT], F32, tag="out", name=f"out{cp}_{nt}")
                nc.vector.tensor_tensor(
                    out=out_sb, in0=st[f"fr{cp}"], in1=st["zb"],
                    op=mybir.AluOpType.mult,
                )
                nc.sync.dma_start(out=outT[cp, :, nsl], in_=out_sb)

            def lag_target(nt, u):
                # after a tile boundary, hold back the new tile's first O'/Z
                # so ~3 T-packs of PE work cover the PSUM-evacuation WAR
                if nt == 0:
                    return 1
                return {1: 2, 2: 3, 3: 3, 4: 3, 5: 2}.get(u, 1)

            pending = []
            tq = {}  # (nt, u) -> emitted-ahead T psum tile

            def emit_T(nt, u):
                # T-packs are emitted one position ahead of their exp/OZ so
                # at tile boundaries the next tile's first T executes before
                # the previous tile's last O'/Z and ScalarE never drains
                g, s0 = u // 2, (2 * u) % 4
                t_ps = pt.tile([128, 2, NT], F32, tag="t", name=f"t{nt}_{u}")
                for j in range(2):
                    s = s0 + j
                    nc.tensor.matmul(
                        t_ps[:, j, :],
                        kt_stack[g][32 * s:32 * (s + 1), :],
                        qts[nt][32 * s:32 * (s + 1), :],
                        start=True, stop=True,
                        tile_position=(32 * s, 0),
                    )
                tq[(nt, u)] = t_ps

            def emit_unit(nt, u):
                t_ps = tq.pop((nt, u))
                e_sb = ep.tile([128, 2, NT], F8, tag="e", name=f"e{nt}_{u}")
                # exp split: ScalarE does cols [0, NT-WD), DVE does the last
                # WD via the fp8e4m3 bit-trick (u8 = round(s*EXPA + EXPB),
                # exact round-to-nearest; zero-mean so softmax stays untilted)
                nc.scalar.activation(
                    out=e_sb[:, :, 0:NT - WD], in_=t_ps[:, :, 0:NT - WD],
                    func=FT.Exp)
                nc.vector.tensor_scalar(
                    out=e_sb[:, :, NT - WD:].bitcast(U8),
                    in0=t_ps[:, :, NT - WD:],
                    scalar1=EXPA, scalar2=EXPB,
                    op0=mybir.AluOpType.mult, op1=mybir.AluOpType.add)
                pending.append((nt, u, e_sb))
                while len(pending) > lag_target(nt, u):
                    pnt, pu, pe = pending.pop(0)
                    emit_oz(pnt, pu, pe)
                    if pu == NU - 1:
                        tail1(pnt)      # right after the O'/Z stop
                if u == 13 and nt + 2 <= NTILES - 1:
                    # Q for tile nt+2, off the critical path
                    qproj(nt + 2, f_psum)
                # deferred tails for the PREVIOUS tile
                if nt > 0:
                    if u == 5:
                        tail2(nt - 1, 0)
                    elif u == 6:
                        tail2(nt - 1, 1)
                    elif u == 8:
                        tail2z(nt - 1)
                    elif u == 10:
                        tail3(nt - 1, 0)
                        tail3(nt - 1, 1)

            # tile 0's units interleave with the tail of the projection chase
            # (K group g / V pairs land 2 rounds ahead of the units that use
            # them, so attention starts as soon as kstk0/qts0 are up)
            # once exp owns ScalarE, stop issuing DMA triggers from it
            dma_engs = [nc.sync, nc.gpsimd]

            sched = []
            for r in range(2, NGRP + 2):
                if r < NGRP:
                    sched.append(("proj", r))
                sched.append(("unit", (0, 2 * (r - 2))))
                sched.append(("unit", (0, 2 * (r - 2) + 1)))
            for nt in range(1, NTILES):
                for u in range(NU):
                    sched.append(("unit", (nt, u)))
            units = [a for k, a in sched if k == "unit"]
            emit_T(*units[0])
            ui = 0
            for kind, arg in sched:
                if kind == "proj":
                    r = arg
                    kproj(r)
                    vpair(2 * r, lambda: proj_psum([128, C]))
                    vpair(2 * r + 1, lambda: proj_psum([128, C]))
                else:
                    ui += 1
                    if ui < len(units):
                        emit_T(*units[ui])
                    emit_unit(*arg)
            for pnt, pu, pe in pending:
                emit_oz(pnt, pu, pe)
                if pu == NU - 1:
                    tail1(pnt)
            for cp in range(2):
                tail2(NTILES - 1, cp)
            tail2z(NTILES - 1)
            for cp in range(2):
                tail3(NTILES - 1, cp)

    _split_multiwaits(nc)
    return nc


def _split_multiwaits(nc: bass.Bass) -> None:
    """This container's walrus accepts at most ONE sync-wait per instruction
    (CoreV3GenImpl setupSyncWait). Tile emits multi-wait instructions; split
    the excess waits onto EventSemaphore carriers inserted just before the
    instruction on the same engine."""
    import json as _json

    data = _json.loads(mybir.module_to_json_bytes(nc.m))
    uid = 0
    for fn in data["functions"]:
        for bb in fn["blocks"]:
            new = []
            for inst in bb["instructions"]:
                si = inst.get("sync_info")
                waits = (si or {}).get("on_wait") or []
                if len(waits) > 1:
                    for wcmd in waits[:-1]:
                        uid += 1
                        new.append({
                            "debug": inst.get("debug", 0),
                            "engine": inst["engine"],
                            "ins": [], "outs": [],
                            "name": f"syncw-{uid}",
                            "opcode": "EventSemaphore",
                            "sync_info": {"on_update": [], "on_wait": [wcmd]},
                        })
                    si["on_wait"] = [waits[-1]]
                new.append(inst)
            bb["instructions"] = new
    nc.m = mybir.module_from_json_bytes(_json.dumps(data).encode())


_NC = None


def _get_nc():
    global _NC
    if _NC is None:
        _NC = build_nc()
    return _NC


def _prep_maps(x, Wf, bf, Wg, bg, Wh, bh, Wo, bo):
    bft = ml_dtypes.bfloat16
    shared = {
        "wf": np.ascontiguousarray(Wf.reshape(2, 128, KEY).astype(bft)),
        "wg": np.ascontiguousarray(Wg.reshape(2, 128, KEY).astype(bft)),
        "wh": np.ascontiguousarray(Wh.reshape(2, 128, C).astype(bft)),
        "wo": np.ascontiguousarray(Wo.reshape(2, 128, C).astype(bft)),
        "bfT": np.ascontiguousarray(bf.reshape(KEY, 1).astype(np.float32)),
        "bgT": np.ascontiguousarray(bg.reshape(KEY, 1).astype(np.float32)),
        "bhp": np.ascontiguousarray(bh.reshape(1, C).astype(bft)),
        "bop": np.ascontiguousarray(bo.reshape(1, C).astype(bft)),
    }
    in_maps = []
    for b in range(B):
        xTb = np.ascontiguousarray(
            x[b].reshape(N, C).T.astype(bft).reshape(2, 128, N)
        )
        m = dict(shared)
        m["xT"] = xTb
        in_maps.append(m)
    return in_maps


def run(x, Wf, bf, Wg, bg, Wh, bh, Wo, bo, trace=False, **kw):
    x = np.asarray(x, dtype=np.float32)
    in_maps = _prep_maps(
        x, *(np.asarray(a, dtype=np.float32) for a in (Wf, bf, Wg, bg, Wh, bh, Wo, bo))
    )
    res = run_bass_kernel_spmd(_get_nc(), in_maps, list(range(B)), trace=trace, **kw)
    out = np.empty((B, H, W, C), dtype=np.float32)
    for b in range(B):
        oT = np.asarray(res.results[b]["outT"], dtype=np.float32).reshape(C, N)
        out[b] = oT.T.reshape(H, W, C)
    return out, res


def kernel(x, Wf, bf, Wg, bg, Wh, bh, Wo, bo):
    out, _ = run(x, Wf, bf, Wg, bg, Wh, bh, Wo, bo)
    return out



# revision 25
# speedup vs baseline: 1.2117x; 1.0239x over previous
"""SAGAN-style self-attention on 8 trn2 cores: data-parallel over batch.

Per core (one batch image): x^T [256,4096] bf16 in, out^T [256,4096] f32 out.
Projections (Q/K/V) chase the x DMAs (round-robin across SP/ACT/Pool
triggers); tile 0's attention units interleave with the tail of that chase.

  QT/KT = W^T @ xT + b        [32, 4096]  (bias via DVE tensor_scalar; K goes
                              straight into per-group kt_stack row strips)
  V     = x @ Wh + bh         fp8e4 pair tiles [128, 2, 256] ([keys, pair, c])
  per 512-query tile, per unit = pair of 128-key blocks (16 units/tile):
    T    = KT_strips.T @ QT    [128 keys, 2*512] 2-way row-packed (K=32),
                               t psum double-buffered so exp(u) || T(u+1)
    expT = exp(T)              ScalarE, PSUM->SBUF, fp8e4 (|s|<~5, no max-sub)
    O'  += V2_pair.T @ E2_pair [256, 512] fp8 DoubleRow PSUM accum
    Z   += ones2.T @ E2_pair   [16, 512] fp8 DoubleRow (rows identical)
  tail (deferred, off the PE critical path; O'/Z of the next tile are held
  back a few units so PSUM-evacuation WARs are covered by T-pack work):
    osb  = O' -> bf16 (DVE, frees o banks early); zf32 = Z -> SBUF (frees zz)
    zr   = 1/Z (DVE reciprocal, reads the SBUF copy)
    f    = bo (x) Z (K=1 preload) + Wo^T @ osb   (normalize AFTER projection:
    fraw = f -> bf16 (DVE, frees the f bank)      the division commutes)
    zb   = ones (x) (1/Z) broadcast [128, 512] (PE, borrows the f slot)
    out  = fraw * zb (DVE) -> DMA
PSUM banks: t=2x2 (double-buffered T), o0/o1=2, zz=1, f=1 -> 8 exactly.
fp8 E/V numerics: rel-L2 vs f64 reference = 1.47e-2 (gate 2e-2).
"""

import sys

if "/opt/trn_rl_repo" not in sys.path:
    sys.path.insert(0, "/opt/trn_rl_repo")

import ml_dtypes
import numpy as np

import concourse.bass as bass
import concourse.mybir as mybir
import concourse.tile as tile
from concourse.bass_utils import run_bass_kernel_spmd

B, H, W, C = 8, 64, 64, 256
KEY = 32
N = H * W          # 4096 tokens
NT = 512           # query tile (free dim per matmul)
NTILES = N // NT   # 8
MB = 128           # key block
NMB = N // MB      # 32
GRP = 4            # key blocks per group (one per PE row strip)
NGRP = NMB // GRP  # 8

BF16 = mybir.dt.bfloat16
F32 = mybir.dt.float32
F8 = mybir.dt.float8e4
U8 = mybir.dt.uint8
FT = mybir.ActivationFunctionType
DR = mybir.MatmulPerfMode.DoubleRow

WD = 0            # exp cols per unit half offloaded to DVE fast-exp
EXPA = 8.0 / float(np.log(2.0))
EXPB = 55.55       # zero-mean fp8e4m3 bit-trick bias (probe: exact u8 RN)


def build_nc() -> bass.Bass:
    nc = bass.Bass()

    xT = nc.declare_dram_parameter("xT", [2, 128, N], BF16, isOutput=False)
    wf = nc.declare_dram_parameter("wf", [2, 128, KEY], BF16, isOutput=False)
    wg = nc.declare_dram_parameter("wg", [2, 128, KEY], BF16, isOutput=False)
    wh = nc.declare_dram_parameter("wh", [2, 128, C], BF16, isOutput=False)
    wo = nc.declare_dram_parameter("wo", [2, 128, C], BF16, isOutput=False)
    bfT = nc.declare_dram_parameter("bfT", [KEY, 1], F32, isOutput=False)
    bgT = nc.declare_dram_parameter("bgT", [KEY, 1], F32, isOutput=False)
    bhp = nc.declare_dram_parameter("bhp", [1, C], BF16, isOutput=False)
    bop = nc.declare_dram_parameter("bop", [1, C], BF16, isOutput=False)
    outT = nc.declare_dram_parameter("outT", [2, 128, N], F32, isOutput=True)

    # per-tile DRAM scratch for the 1/Z reshape round-trip: [1,512] on one
    # partition -> [128,4] so the DVE reciprocal runs 128 lanes wide
    zdram = [nc.dram_tensor(f"zdram{i}", [1, NT], BF16, kind="Internal")
             for i in range(NTILES)]
    rdram = [nc.dram_tensor(f"rdram{i}", [128, NT // 128], BF16, kind="Internal")
             for i in range(NTILES)]

    with tile.TileContext(nc) as tc:
        with (
            tc.tile_pool(name="const", bufs=1) as const,
            tc.tile_pool(name="xp", bufs=1) as xp,
            tc.tile_pool(name="vp", bufs=1) as vp,
            tc.tile_pool(name="qk", bufs=1) as qk,
            tc.tile_pool(name="ep", bufs=5) as ep,
            tc.tile_pool(name="osb", bufs=2) as osbp,
            tc.tile_pool(name="frp", bufs=2) as frp,
            tc.tile_pool(name="zsp", bufs=2) as zsp,
            tc.tile_pool(name="outp", bufs=3) as outp,
            tc.tile_pool(name="pt", bufs=2, space="PSUM") as pt,
            tc.tile_pool(name="po", bufs=1, space="PSUM") as po,
            tc.tile_pool(name="pz", bufs=1, space="PSUM") as pz,
            tc.tile_pool(name="pf", bufs=1, space="PSUM") as pf,
        ):
            # ---- constants ----
            ones2 = const.tile([128, 2, 16], F8)    # Z DoubleRow lhsT
            # (16-wide: DR weights need pair-step %16B == 0; rows identical)
            ones_m = const.tile([1, 128], BF16)     # K=1 broadcast lhsT
            nc.vector.memset(ones2, 1.0)
            nc.vector.memset(ones_m, 1.0)

            wf_sb = const.tile([128, 2, KEY], BF16)
            wg_sb = const.tile([128, 2, KEY], BF16)
            wh_sb = const.tile([128, 2, C], BF16)
            wo_sb = const.tile([128, 2, C], BF16)
            bf_sb = const.tile([KEY, 1], F32)
            bg_sb = const.tile([KEY, 1], F32)
            bh_sb = const.tile([1, C], BF16)
            bo_sb = const.tile([1, C], BF16)
            # DMA trigger round-robin across the three HWDGE/SWDGE engines:
            # each trigger costs ~600ns of that engine's sequencer, and a
            # single queue would serialize the whole input load (~40us).
            dma_i = 0
            dma_engs = [nc.sync, nc.scalar, nc.gpsimd]

            def dma_rr(out, in_):
                nonlocal dma_i
                eng = dma_engs[dma_i % len(dma_engs)]
                dma_i += 1
                eng.dma_start(out=out, in_=in_)
            # bh broadcast to all 128 partitions (for V bias add on DVE),
            # via K=1 ones matmul into a borrowed PSUM slot
            bh_bc = const.tile([128, C], BF16)

            # xT chunks in 512-col tiles (projections start on first slice)
            xts = [
                [xp.tile([128, NT], BF16, name=f"xt{cc}_{h}") for h in range(NTILES)]
                for cc in range(2)
            ]
            # x chunks first (K proj group g needs x tile h=g), weights
            # interleaved right behind the first x pair
            def xdma(h):
                for cc in range(2):
                    dma_rr(xts[cc][h], xT[cc, :, h * NT:(h + 1) * NT])

            xdma(0)
            for cc in range(2):
                dma_rr(wg_sb[:, cc, :], wg[cc])
                dma_rr(wf_sb[:, cc, :], wf[cc])
            dma_rr(bg_sb, bgT[:])
            dma_rr(bf_sb, bfT[:])
            xdma(1)
            for cc in range(2):
                dma_rr(wh_sb[:, cc, :], wh[cc])
            dma_rr(bh_sb, bhp[:])
            xdma(2)
            for cc in range(2):
                dma_rr(wo_sb[:, cc, :], wo[cc])
            dma_rr(bo_sb, bop[:])
            for h in range(3, NTILES):
                xdma(h)

            def xs(cc, start, width):
                h = start // NT
                assert (start + width - 1) // NT == h
                return xts[cc][h][:, start - h * NT: start - h * NT + width]

            pp_i = 0

            def proj_psum(shape):
                # borrow the "t"/"f" slots (alternating) before attention
                nonlocal pp_i
                pp_i += 1
                if pp_i % 2:
                    return pt.tile(shape, F32, tag="t", name=f"projps{pp_i}")
                return pf.tile(shape, F32, tag="f", name=f"projps{pp_i}")

            bh_ps = proj_psum([128, C])
            nc.tensor.matmul(bh_ps, ones_m, bh_sb, start=True, stop=True)
            nc.vector.tensor_copy(out=bh_bc, in_=bh_ps)

            # ---- K projection; per-group kt_stack tiles (so T of group g
            # only waits for group g), regrouped into row strips by
            # partition-shifting SBUF->SBUF DMAs
            kt_stack = [
                qk.tile([128, MB], BF16, name=f"kstk{g}") for g in range(NGRP)
            ]

            def kproj(g):
                ps = proj_psum([KEY, NT])
                for cc in range(2):
                    nc.tensor.matmul(
                        ps, wg_sb[:, cc, :], xs(cc, g * NT, NT),
                        start=(cc == 0), stop=(cc == 1),
                    )
                ktg = qk.tile([KEY, NT], BF16, name=f"kts{g}")
                nc.vector.tensor_scalar_add(ktg, ps, bg_sb)
                for i in range(4):
                    dma_rr(
                        kt_stack[g][32 * i:32 * (i + 1), :],
                        ktg[:, i * MB:(i + 1) * MB],
                    )

            # ---- per-query-tile Q tiles (replicated into 4 row strips) -----
            qts = [
                qk.tile([128, NT], BF16, name=f"qts{nt}") for nt in range(NTILES)
            ]

            def f_psum(shape):
                nonlocal pp_i
                pp_i += 1
                return pf.tile(shape, F32, tag="f", name=f"fps{pp_i}")

            def qproj(nt, psum_fn=None):
                ps = (psum_fn or proj_psum)([KEY, NT])
                for cc in range(2):
                    nc.tensor.matmul(
                        ps, wf_sb[:, cc, :], xs(cc, nt * NT, NT),
                        start=(cc == 0), stop=(cc == 1),
                    )
                nc.vector.tensor_scalar_add(qts[nt][0:KEY, :], ps, bf_sb)
                for i in range(1, 4):
                    nc.sync.dma_start(
                        out=qts[nt][32 * i:32 * (i + 1), :], in_=qts[nt][0:KEY, :]
                    )

            # ---- V projection -> fp8 pair tiles [128, 2, C] ----------------
            v2 = [
                vp.tile([128, 2, C], F8, tag=f"v{p}", name=f"v{p}")
                for p in range(NMB // 2)
            ]

            def vpair(pair, psum_fn):
                for mem in range(2):
                    mb = 2 * pair + mem
                    ps = psum_fn()
                    for cc in range(2):
                        nc.tensor.matmul(
                            ps, xs(cc, mb * MB, MB), wh_sb[:, cc, :],
                            start=(cc == 0), stop=(cc == 1),
                        )
                    nc.vector.tensor_tensor(
                        out=v2[pair][:, mem, :], in0=ps, in1=bh_bc,
                        op=mybir.AluOpType.add,
                    )

            # prologue: Q for tiles 0/1 and the first two K groups / four V
            # pairs; K groups 2..7 (+ V pairs) interleave with tile 0's units
            qproj(0)
            qproj(1)
            for g in range(2):
                kproj(g)
                vpair(2 * g, lambda: proj_psum([128, C]))
                vpair(2 * g + 1, lambda: proj_psum([128, C]))

            # ---- attention: pipelined over (query-tile, pair-unit) ----------
            # unit u = one pair of key blocks (2u, 2u+1); 16 units per tile.
            # T psum is [128, 2*NT] (2 banks) double-buffered so exp(u) and
            # T(u+1) overlap; row strips alternate (0,1)/(2,3) across units.
            NU = NMB // 2  # 16
            state = {}  # nt -> dict with live tiles for the tail

            def emit_oz(nt, u, e_sb):
                if u == 0:
                    state[nt] = {
                        "o": [po.tile([128, NT], F32, tag="o0", name=f"o0_{nt}"),
                              po.tile([128, NT], F32, tag="o1", name=f"o1_{nt}")],
                        "zz": pz.tile([16, NT], F32, tag="z", name=f"z{nt}"),
                    }
                st = state[nt]
                first, last = u == 0, u == NU - 1
                for cc in range(2):
                    nc.tensor.matmul(
                        st["o"][cc],
                        v2[u][:, :, cc * 128:(cc + 1) * 128],
                        e_sb,
                        start=first, stop=last,
                        perf_mode=DR,
                    )
                nc.tensor.matmul(
                    st["zz"], ones2, e_sb,
                    start=first, stop=last,
                    perf_mode=DR,
                )

            def tail1(nt):
                """PSUM evacuation; emit BEFORE next tile's first O'/Z.
                zz is freed by one fast copy (the slow reciprocal reads
                the SBUF copy later)."""
                st = state[nt]
                ot0 = osbp.tile([128, NT], BF16, tag="os0", name=f"os0_{nt}")
                nc.vector.tensor_copy(out=ot0, in_=st["o"][0])
                ot1 = osbp.tile([128, NT], BF16, tag="os1", name=f"os1_{nt}")
                nc.vector.tensor_copy(out=ot1, in_=st["o"][1])
                st["osb0"], st["osb1"] = ot0, ot1
                zbf = zsp.tile([1, NT], BF16, tag="zbf", name=f"zbf{nt}")
                nc.vector.tensor_copy(out=zbf, in_=st["zz"][0:1, :])
                st["zbf"] = zbf
                # 1/Z via DRAM round-trip reshape: the [1,512] vector lives on
                # one partition (DVE reciprocal would run on 1 lane, 8 cyc/elem
                # = 3.3us); bounce it through DRAM as [128,4] instead. All 4
                # DMAs on the sync queue so they execute in order; the chain
                # has ~8 units of slack before tail2z consumes zrb.
                nc.sync.dma_start(out=zdram[nt][:], in_=zbf)
                zs = zsp.tile([128, NT // 128], BF16, tag="zs", name=f"zs{nt}")
                nc.sync.dma_start(
                    out=zs, in_=zdram[nt][:].rearrange("i (p f) -> (i p) f", p=128))
                zr = zsp.tile([128, NT // 128], F32, tag="zr", name=f"zr{nt}")
                nc.vector.reciprocal(out=zr, in_=zs)
                zrb = zsp.tile([128, NT // 128], BF16, tag="zrb", name=f"zrb{nt}")
                nc.vector.tensor_copy(out=zrb, in_=zr)
                nc.sync.dma_start(out=rdram[nt][:], in_=zrb)
                zrbp = zsp.tile([1, NT], BF16, tag="zrbp", name=f"zrbp{nt}")
                nc.sync.dma_start(
                    out=zrbp, in_=rdram[nt][:].rearrange("p f -> () (p f)"))
                st["zrb"] = zrbp

            def tail2(nt, cp):
                """out-proj half cp: f = bo (x) Z + Wo^T @ osb; fraw; defer mul."""
                st = state[nt]
                csl = slice(cp * 128, (cp + 1) * 128)
                f_ps = pf.tile([128, NT], F32, tag="f", name=f"f{cp}_{nt}")
                nc.tensor.matmul(
                    f_ps, bo_sb[:, csl], st["zbf"], start=True, stop=False,
                )
                for cc in range(2):
                    nc.tensor.matmul(
                        f_ps, wo_sb[:, cc, csl], st[f"osb{cc}"],
                        start=False, stop=(cc == 1),
                    )
                fr = frp.tile([128, NT], BF16, tag=f"fr{cp}", name=f"fr{cp}_{nt}")
                nc.vector.tensor_copy(out=fr, in_=f_ps)
                st[f"fr{cp}"] = fr

            def tail2z(nt):
                """1/Z broadcast to 128 partitions (borrows the f slot);
                deferred past the reciprocal's latency."""
                st = state[nt]
                zb_ps = pf.tile([128, NT], F32, tag="f", name=f"zbp{nt}")
                nc.tensor.matmul(zb_ps, ones_m, st["zrb"], start=True, stop=True)
                zb = zsp.tile([128, NT], BF16, tag="zb", name=f"zb{nt}")
                nc.vector.tensor_copy(out=zb, in_=zb_ps)
                st["zb"] = zb

            def tail3(nt, cp):
                st = state[nt]
                nsl = slice(nt * NT, (nt + 1) * NT)
                out_sb = outp.tile([128, NT], F32, tag="out", name=f"out{cp}_{nt}")
                nc.vector.tensor_tensor(
                    out=out_sb, in0=st[f"fr{cp}"], in1=st["zb"],
                    op=mybir.AluOpType.mult,
                )
                nc.sync.dma_start(out=outT[cp, :, nsl], in_=out_sb)

            def lag_target(nt, u):
                # after a tile boundary, hold back the new tile's first O'/Z
                # so ~3 T-packs of PE work cover the PSUM-evacuation WAR
                if nt == 0:
                    return 1
                return {1: 2, 2: 3, 3: 3, 4: 3, 5: 2}.get(u, 1)

            pending = []
            tq = {}  # (nt, u) -> emitted-ahead T psum tile

            def emit_T(nt, u):
                # T-packs are emitted one position ahead of their exp/OZ so
                # at tile boundaries the next tile's first T executes before
                # the previous tile's last O'/Z and ScalarE never drains
                g, s0 = u // 2, (2 * u) % 4
                t_ps = pt.tile([128, 2, NT], F32, tag="t", name=f"t{nt}_{u}")
                for j in range(2):
                    s = s0 + j
                    nc.tensor.matmul(
                        t_ps[:, j, :],
                        kt_stack[g][32 * s:32 * (s + 1), :],
                        qts[nt][32 * s:32 * (s + 1), :],
                        start=True, stop=True,
                        tile_position=(32 * s, 0),
                    )
                tq[(nt, u)] = t_ps

            def emit_unit(nt, u):
                t_ps = tq.pop((nt, u))
                e_sb = ep.tile([128, 2, NT], F8, tag="e", name=f"e{nt}_{u}")
                # exp split: ScalarE does cols [0, NT-WD), DVE does the last
                # WD via the fp8e4m3 bit-trick (u8 = round(s*EXPA + EXPB),
                # exact round-to-nearest; zero-mean so softmax stays untilted)
                nc.scalar.activation(
                    out=e_sb[:, :, 0:NT - WD], in_=t_ps[:, :, 0:NT - WD],
                    func=FT.Exp)
                if WD:
                    nc.vector.tensor_scalar(
                        out=e_sb[:, :, NT - WD:].bitcast(U8),
                        in0=t_ps[:, :, NT - WD:],
                        scalar1=EXPA, scalar2=EXPB,
                        op0=mybir.AluOpType.mult, op1=mybir.AluOpType.add)
                pending.append((nt, u, e_sb))
                while len(pending) > lag_target(nt, u):
                    pnt, pu, pe = pending.pop(0)
                    emit_oz(pnt, pu, pe)
                    if pu == NU - 1:
                        tail1(pnt)      # right after the O'/Z stop
                if u == 13 and nt + 2 <= NTILES - 1:
                    # Q for tile nt+2, off the critical path
                    qproj(nt + 2, f_psum)
                # deferred tails for the PREVIOUS tile
                if nt > 0:
                    if u == 5:
                        tail2(nt - 1, 0)
                    elif u == 6:
                        tail2(nt - 1, 1)
                    elif u == 8:
                        tail2z(nt - 1)
                    elif u == 10:
                        tail3(nt - 1, 0)
                        tail3(nt - 1, 1)

            # tile 0's units interleave with the tail of the projection chase
            # (K group g / V pairs land 2 rounds ahead of the units that use
            # them, so attention starts as soon as kstk0/qts0 are up)
            # once exp owns ScalarE, stop issuing DMA triggers from it
            dma_engs = [nc.sync, nc.gpsimd]

            sched = []
            for r in range(2, NGRP + 2):
                if r < NGRP:
                    sched.append(("proj", r))
                sched.append(("unit", (0, 2 * (r - 2))))
                sched.append(("unit", (0, 2 * (r - 2) + 1)))
            for nt in range(1, NTILES):
                for u in range(NU):
                    sched.append(("unit", (nt, u)))
            units = [a for k, a in sched if k == "unit"]
            emit_T(*units[0])
            ui = 0
            for kind, arg in sched:
                if kind == "proj":
                    r = arg
                    kproj(r)
                    vpair(2 * r, lambda: proj_psum([128, C]))
                    vpair(2 * r + 1, lambda: proj_psum([128, C]))
                else:
                    ui += 1
                    if ui < len(units):
                        emit_T(*units[ui])
                    emit_unit(*arg)
            for pnt, pu, pe in pending:
                emit_oz(pnt, pu, pe)
                if pu == NU - 1:
                    tail1(pnt)
            for cp in range(2):
                tail2(NTILES - 1, cp)
            tail2z(NTILES - 1)
            for cp in range(2):
                tail3(NTILES - 1, cp)

    _split_multiwaits(nc)
    return nc


def _split_multiwaits(nc: bass.Bass) -> None:
    """This container's walrus accepts at most ONE sync-wait per instruction
    (CoreV3GenImpl setupSyncWait). Tile emits multi-wait instructions; split
    the excess waits onto EventSemaphore carriers inserted just before the
    instruction on the same engine."""
    import json as _json

    data = _json.loads(mybir.module_to_json_bytes(nc.m))
    uid = 0
    for fn in data["functions"]:
        for bb in fn["blocks"]:
            new = []
            for inst in bb["instructions"]:
                si = inst.get("sync_info")
                waits = (si or {}).get("on_wait") or []
                if len(waits) > 1:
                    for wcmd in waits[:-1]:
                        uid += 1
                        new.append({
                            "debug": inst.get("debug", 0),
                            "engine": inst["engine"],
                            "ins": [], "outs": [],
                            "name": f"syncw-{uid}",
                            "opcode": "EventSemaphore",
                            "sync_info": {"on_update": [], "on_wait": [wcmd]},
                        })
                    si["on_wait"] = [waits[-1]]
                new.append(inst)
            bb["instructions"] = new
    nc.m = mybir.module_from_json_bytes(_json.dumps(data).encode())


_NC = None


def _get_nc():
    global _NC
    if _NC is None:
        _NC = build_nc()
    return _NC


def _prep_maps(x, Wf, bf, Wg, bg, Wh, bh, Wo, bo):
    bft = ml_dtypes.bfloat16
    shared = {
        "wf": np.ascontiguousarray(Wf.reshape(2, 128, KEY).astype(bft)),
        "wg": np.ascontiguousarray(Wg.reshape(2, 128, KEY).astype(bft)),
        "wh": np.ascontiguousarray(Wh.reshape(2, 128, C).astype(bft)),
        "wo": np.ascontiguousarray(Wo.reshape(2, 128, C).astype(bft)),
        "bfT": np.ascontiguousarray(bf.reshape(KEY, 1).astype(np.float32)),
        "bgT": np.ascontiguousarray(bg.reshape(KEY, 1).astype(np.float32)),
        "bhp": np.ascontiguousarray(bh.reshape(1, C).astype(bft)),
        "bop": np.ascontiguousarray(bo.reshape(1, C).astype(bft)),
    }
    in_maps = []
    for b in range(B):
        xTb = np.ascontiguousarray(
            x[b].reshape(N, C).T.astype(bft).reshape(2, 128, N)
        )
        m = dict(shared)
        m["xT"] = xTb
        in_maps.append(m)
    return in_maps


def run(x, Wf, bf, Wg, bg, Wh, bh, Wo, bo, trace=False, **kw):
    x = np.asarray(x, dtype=np.float32)
    in_maps = _prep_maps(
        x, *(np.asarray(a, dtype=np.float32) for a in (Wf, bf, Wg, bg, Wh, bh, Wo, bo))
    )
    res = run_bass_kernel_spmd(_get_nc(), in_maps, list(range(B)), trace=trace, **kw)
    out = np.empty((B, H, W, C), dtype=np.float32)
    for b in range(B):
        oT = np.asarray(res.results[b]["outT"], dtype=np.float32).reshape(C, N)
        out[b] = oT.T.reshape(H, W, C)
    return out, res


def kernel(x, Wf, bf, Wg, bg, Wh, bh, Wo, bo):
    out, _ = run(x, Wf, bf, Wg, bg, Wh, bh, Wo, bo)
    return out



# revision 26
# speedup vs baseline: 1.2567x; 1.0372x over previous
"""SAGAN-style self-attention on 8 trn2 cores: data-parallel over batch.

Per core (one batch image): x^T [256,4096] bf16 in, out^T [256,4096] f32 out.
Projections (Q/K/V) chase the x DMAs (round-robin across SP/ACT/Pool
triggers); tile 0's attention units interleave with the tail of that chase.

  QT/KT = W^T @ xT + b        [32, 4096]  (bias via DVE tensor_scalar; K goes
                              straight into per-group kt_stack row strips)
  V     = x @ Wh + bh         fp8e4 pair tiles [128, 2, 256] ([keys, pair, c])
  per 512-query tile, per unit = pair of 128-key blocks (16 units/tile):
    T    = KT_strips.T @ QT    [128 keys, 2*512] 2-way row-packed (K=32),
                               t psum double-buffered so exp(u) || T(u+1)
    expT = exp(T)              ScalarE, PSUM->SBUF, fp8e4 (|s|<~5, no max-sub)
    O'  += V2_pair.T @ E2_pair [256, 512] fp8 DoubleRow PSUM accum
    Z   += ones2.T @ E2_pair   [16, 512] fp8 DoubleRow (rows identical)
  tail (deferred, off the PE critical path; O'/Z of the next tile are held
  back a few units so PSUM-evacuation WARs are covered by T-pack work):
    osb  = O' -> bf16 (DVE, frees o banks early); zf32 = Z -> SBUF (frees zz)
    zr   = 1/Z (DVE reciprocal, reads the SBUF copy)
    f    = bo (x) Z (K=1 preload) + Wo^T @ osb   (normalize AFTER projection:
    fraw = f -> bf16 (DVE, frees the f bank)      the division commutes)
    zb   = ones (x) (1/Z) broadcast [128, 512] (PE, borrows the f slot)
    out  = fraw * zb (DVE) -> DMA
PSUM banks: t=2x2 (double-buffered T), o0/o1=2, zz=1, f=1 -> 8 exactly.
fp8 E/V numerics: rel-L2 vs f64 reference = 1.47e-2 (gate 2e-2).
"""

import sys

if "/opt/trn_rl_repo" not in sys.path:
    sys.path.insert(0, "/opt/trn_rl_repo")

import ml_dtypes
import numpy as np

import concourse.bass as bass
import concourse.mybir as mybir
import concourse.tile as tile
from concourse.bass_utils import run_bass_kernel_spmd

B, H, W, C = 8, 64, 64, 256
KEY = 32
N = H * W          # 4096 tokens
NT = 512           # query tile (free dim per matmul)
NTILES = N // NT   # 8
MB = 128           # key block
NMB = N // MB      # 32
GRP = 4            # key blocks per group (one per PE row strip)
NGRP = NMB // GRP  # 8

BF16 = mybir.dt.bfloat16
F32 = mybir.dt.float32
F8 = mybir.dt.float8e4
U8 = mybir.dt.uint8
FT = mybir.ActivationFunctionType
DR = mybir.MatmulPerfMode.DoubleRow

WD = 0            # exp cols per unit half offloaded to DVE fast-exp
EXPA = 8.0 / float(np.log(2.0))
EXPB = 55.55       # zero-mean fp8e4m3 bit-trick bias (probe: exact u8 RN)


def build_nc() -> bass.Bass:
    nc = bass.Bass()

    xT = nc.declare_dram_parameter("xT", [2, 128, N], BF16, isOutput=False)
    wf = nc.declare_dram_parameter("wf", [2, 128, KEY], BF16, isOutput=False)
    wg = nc.declare_dram_parameter("wg", [2, 128, KEY], BF16, isOutput=False)
    wh = nc.declare_dram_parameter("wh", [2, 128, C], BF16, isOutput=False)
    wo = nc.declare_dram_parameter("wo", [2, 128, C], BF16, isOutput=False)
    bfT = nc.declare_dram_parameter("bfT", [KEY, 1], F32, isOutput=False)
    bgT = nc.declare_dram_parameter("bgT", [KEY, 1], F32, isOutput=False)
    bhp = nc.declare_dram_parameter("bhp", [1, C], BF16, isOutput=False)
    bop = nc.declare_dram_parameter("bop", [1, C], BF16, isOutput=False)
    outT = nc.declare_dram_parameter("outT", [2, 128, N], F32, isOutput=True)

    # per-tile DRAM scratch for the 1/Z reshape round-trip: [1,512] on one
    # partition -> [128,4] so the DVE reciprocal runs 128 lanes wide
    zdram = [nc.dram_tensor(f"zdram{i}", [1, NT], BF16, kind="Internal")
             for i in range(NTILES)]
    rdram = [nc.dram_tensor(f"rdram{i}", [128, NT // 128], BF16, kind="Internal")
             for i in range(NTILES)]

    with tile.TileContext(nc) as tc:
        with (
            tc.tile_pool(name="const", bufs=1) as const,
            tc.tile_pool(name="xp", bufs=1) as xp,
            tc.tile_pool(name="vp", bufs=1) as vp,
            tc.tile_pool(name="qk", bufs=1) as qk,
            tc.tile_pool(name="ep", bufs=5) as ep,
            tc.tile_pool(name="osb", bufs=2) as osbp,
            tc.tile_pool(name="frp", bufs=2) as frp,
            tc.tile_pool(name="zsp", bufs=2) as zsp,
            tc.tile_pool(name="outp", bufs=3) as outp,
            tc.tile_pool(name="pt", bufs=2, space="PSUM") as pt,
            tc.tile_pool(name="po", bufs=1, space="PSUM") as po,
            tc.tile_pool(name="pz", bufs=1, space="PSUM") as pz,
            tc.tile_pool(name="pf", bufs=1, space="PSUM") as pf,
        ):
            # ---- constants ----
            ones2 = const.tile([128, 2, 16], F8)    # Z DoubleRow lhsT
            # (16-wide: DR weights need pair-step %16B == 0; rows identical)
            ones_m = const.tile([1, 128], BF16)     # K=1 broadcast lhsT
            nc.vector.memset(ones2, 1.0)
            nc.vector.memset(ones_m, 1.0)

            wf_sb = const.tile([128, 2, KEY], BF16)
            wg_sb = const.tile([128, 2, KEY], BF16)
            wh_sb = const.tile([128, 2, C], BF16)
            wo_sb = const.tile([128, 2, C], BF16)
            bf_sb = const.tile([KEY, 1], F32)
            bg_sb = const.tile([KEY, 1], F32)
            bh_sb = const.tile([1, C], BF16)
            bo_sb = const.tile([1, C], BF16)
            # DMA trigger round-robin across the three HWDGE/SWDGE engines:
            # each trigger costs ~600ns of that engine's sequencer, and a
            # single queue would serialize the whole input load (~40us).
            dma_i = 0
            dma_engs = [nc.sync, nc.scalar, nc.gpsimd]

            def dma_rr(out, in_):
                nonlocal dma_i
                eng = dma_engs[dma_i % len(dma_engs)]
                dma_i += 1
                eng.dma_start(out=out, in_=in_)
            # bh broadcast to all 128 partitions (for V bias add on DVE),
            # via K=1 ones matmul into a borrowed PSUM slot
            bh_bc = const.tile([128, C], BF16)

            # xT chunks in 512-col tiles (projections start on first slice)
            xts = [
                [xp.tile([128, NT], BF16, name=f"xt{cc}_{h}") for h in range(NTILES)]
                for cc in range(2)
            ]
            # x chunks first (K proj group g needs x tile h=g), weights
            # interleaved right behind the first x pair
            def xdma(h):
                for cc in range(2):
                    dma_rr(xts[cc][h], xT[cc, :, h * NT:(h + 1) * NT])

            xdma(0)
            for cc in range(2):
                dma_rr(wg_sb[:, cc, :], wg[cc])
                dma_rr(wf_sb[:, cc, :], wf[cc])
            dma_rr(bg_sb, bgT[:])
            dma_rr(bf_sb, bfT[:])
            xdma(1)
            for cc in range(2):
                dma_rr(wh_sb[:, cc, :], wh[cc])
            dma_rr(bh_sb, bhp[:])
            xdma(2)
            for cc in range(2):
                dma_rr(wo_sb[:, cc, :], wo[cc])
            dma_rr(bo_sb, bop[:])
            for h in range(3, NTILES):
                xdma(h)

            def xs(cc, start, width):
                h = start // NT
                assert (start + width - 1) // NT == h
                return xts[cc][h][:, start - h * NT: start - h * NT + width]

            pp_i = 0

            def proj_psum(shape):
                # borrow the "t"/"f" slots (alternating) before attention
                nonlocal pp_i
                pp_i += 1
                if pp_i % 2:
                    return pt.tile(shape, F32, tag="t", name=f"projps{pp_i}")
                return pf.tile(shape, F32, tag="f", name=f"projps{pp_i}")

            bh_ps = proj_psum([128, C])
            nc.tensor.matmul(bh_ps, ones_m, bh_sb, start=True, stop=True)
            nc.vector.tensor_copy(out=bh_bc, in_=bh_ps)

            # ---- K projection; per-group kt_stack tiles (so T of group g
            # only waits for group g), regrouped into row strips by
            # partition-shifting SBUF->SBUF DMAs
            kt_stack = [
                qk.tile([128, MB], BF16, name=f"kstk{g}") for g in range(NGRP)
            ]

            def kproj(g):
                ps = proj_psum([KEY, NT])
                for cc in range(2):
                    nc.tensor.matmul(
                        ps, wg_sb[:, cc, :], xs(cc, g * NT, NT),
                        start=(cc == 0), stop=(cc == 1),
                    )
                ktg = qk.tile([KEY, NT], BF16, name=f"kts{g}")
                nc.vector.tensor_scalar_add(ktg, ps, bg_sb)
                for i in range(4):
                    dma_rr(
                        kt_stack[g][32 * i:32 * (i + 1), :],
                        ktg[:, i * MB:(i + 1) * MB],
                    )

            # ---- per-query-tile Q tiles (replicated into 4 row strips) -----
            qts = [
                qk.tile([128, NT], BF16, name=f"qts{nt}") for nt in range(NTILES)
            ]

            def f_psum(shape):
                nonlocal pp_i
                pp_i += 1
                return pf.tile(shape, F32, tag="f", name=f"fps{pp_i}")

            def qproj(nt, psum_fn=None):
                ps = (psum_fn or proj_psum)([KEY, NT])
                for cc in range(2):
                    nc.tensor.matmul(
                        ps, wf_sb[:, cc, :], xs(cc, nt * NT, NT),
                        start=(cc == 0), stop=(cc == 1),
                    )
                nc.vector.tensor_scalar_add(qts[nt][0:KEY, :], ps, bf_sb)
                for i in range(1, 4):
                    nc.sync.dma_start(
                        out=qts[nt][32 * i:32 * (i + 1), :], in_=qts[nt][0:KEY, :]
                    )

            # ---- V projection -> fp8 pair tiles [128, 2, C] ----------------
            v2 = [
                vp.tile([128, 2, C], F8, tag=f"v{p}", name=f"v{p}")
                for p in range(NMB // 2)
            ]

            def vpair(pair, psum_fn):
                for mem in range(2):
                    mb = 2 * pair + mem
                    ps = psum_fn()
                    for cc in range(2):
                        nc.tensor.matmul(
                            ps, xs(cc, mb * MB, MB), wh_sb[:, cc, :],
                            start=(cc == 0), stop=(cc == 1),
                        )
                    nc.vector.tensor_tensor(
                        out=v2[pair][:, mem, :], in0=ps, in1=bh_bc,
                        op=mybir.AluOpType.add,
                    )

            # prologue: Q for tiles 0/1 and the first two K groups / four V
            # pairs; K groups 2..7 (+ V pairs) interleave with tile 0's units
            qproj(0)
            qproj(1)
            for g in range(2):
                kproj(g)
                vpair(2 * g, lambda: proj_psum([128, C]))
                vpair(2 * g + 1, lambda: proj_psum([128, C]))

            # ---- attention: pipelined over (query-tile, pair-unit) ----------
            # unit u = one pair of key blocks (2u, 2u+1); 16 units per tile.
            # T psum is [128, 2*NT] (2 banks) double-buffered so exp(u) and
            # T(u+1) overlap; row strips alternate (0,1)/(2,3) across units.
            NU = NMB // 2  # 16
            state = {}  # nt -> dict with live tiles for the tail

            def emit_oz(nt, u, e_sb):
                if u == 0:
                    state[nt] = {
                        "o": [po.tile([128, NT], F32, tag="o0", name=f"o0_{nt}"),
                              po.tile([128, NT], F32, tag="o1", name=f"o1_{nt}")],
                        "zz": pz.tile([16, NT], F32, tag="z", name=f"z{nt}"),
                    }
                st = state[nt]
                first, last = u == 0, u == NU - 1
                for cc in range(2):
                    nc.tensor.matmul(
                        st["o"][cc],
                        v2[u][:, :, cc * 128:(cc + 1) * 128],
                        e_sb,
                        start=first, stop=last,
                        perf_mode=DR,
                    )
                nc.tensor.matmul(
                    st["zz"], ones2, e_sb,
                    start=first, stop=last,
                    perf_mode=DR,
                )

            def tail1(nt):
                """PSUM evacuation; emit BEFORE next tile's first O'/Z.
                zz is freed by one fast copy (the slow reciprocal reads
                the SBUF copy later)."""
                st = state[nt]
                ot0 = osbp.tile([128, NT], BF16, tag="os0", name=f"os0_{nt}")
                nc.vector.tensor_copy(out=ot0, in_=st["o"][0])
                ot1 = osbp.tile([128, NT], BF16, tag="os1", name=f"os1_{nt}")
                nc.vector.tensor_copy(out=ot1, in_=st["o"][1])
                st["osb0"], st["osb1"] = ot0, ot1
                zf32 = zsp.tile([1, NT], F32, tag="zf32", name=f"zf32_{nt}")
                nc.vector.tensor_copy(out=zf32, in_=st["zz"][0:1, :])
                zbf = zsp.tile([1, NT], BF16, tag="zbf", name=f"zbf{nt}")
                nc.vector.tensor_copy(out=zbf, in_=zf32)
                zr = zsp.tile([1, NT], F32, tag="zr", name=f"zr{nt}")
                nc.vector.reciprocal(out=zr, in_=zf32)
                zrb = zsp.tile([1, NT], BF16, tag="zrb", name=f"zrb{nt}")
                nc.vector.tensor_copy(out=zrb, in_=zr)
                st["zbf"], st["zrb"] = zbf, zrb

            def tail2(nt, cp):
                """out-proj half cp: f = bo (x) Z + Wo^T @ osb; fraw; defer mul."""
                st = state[nt]
                csl = slice(cp * 128, (cp + 1) * 128)
                f_ps = pf.tile([128, NT], F32, tag="f", name=f"f{cp}_{nt}")
                nc.tensor.matmul(
                    f_ps, bo_sb[:, csl], st["zbf"], start=True, stop=False,
                )
                for cc in range(2):
                    nc.tensor.matmul(
                        f_ps, wo_sb[:, cc, csl], st[f"osb{cc}"],
                        start=False, stop=(cc == 1),
                    )
                fr = frp.tile([128, NT], BF16, tag=f"fr{cp}", name=f"fr{cp}_{nt}")
                nc.vector.tensor_copy(out=fr, in_=f_ps)
                st[f"fr{cp}"] = fr

            def tail2z(nt):
                """1/Z broadcast to 128 partitions (borrows the f slot);
                deferred past the reciprocal's latency."""
                st = state[nt]
                zb_ps = pf.tile([128, NT], F32, tag="f", name=f"zbp{nt}")
                nc.tensor.matmul(zb_ps, ones_m, st["zrb"], start=True, stop=True)
                zb = zsp.tile([128, NT], BF16, tag="zb", name=f"zb{nt}")
                nc.vector.tensor_copy(out=zb, in_=zb_ps)
                st["zb"] = zb

            def tail3(nt, cp):
                st = state[nt]
                nsl = slice(nt * NT, (nt + 1) * NT)
                out_sb = outp.tile([128, NT], F32, tag="out", name=f"out{cp}_{nt}")
                nc.vector.tensor_tensor(
                    out=out_sb, in0=st[f"fr{cp}"], in1=st["zb"],
                    op=mybir.AluOpType.mult,
                )
                nc.sync.dma_start(out=outT[cp, :, nsl], in_=out_sb)

            def lag_target(nt, u):
                # after a tile boundary, hold back the new tile's first O'/Z
                # so ~3 T-packs of PE work cover the PSUM-evacuation WAR
                if nt == 0:
                    return 1
                return {1: 2, 2: 3, 3: 3, 4: 3, 5: 2}.get(u, 1)

            pending = []
            tq = {}  # (nt, u) -> emitted-ahead T psum tile

            def emit_T(nt, u):
                # T-packs are emitted one position ahead of their exp/OZ so
                # at tile boundaries the next tile's first T executes before
                # the previous tile's last O'/Z and ScalarE never drains
                g, s0 = u // 2, (2 * u) % 4
                t_ps = pt.tile([128, 2, NT], F32, tag="t", name=f"t{nt}_{u}")
                for j in range(2):
                    s = s0 + j
                    nc.tensor.matmul(
                        t_ps[:, j, :],
                        kt_stack[g][32 * s:32 * (s + 1), :],
                        qts[nt][32 * s:32 * (s + 1), :],
                        start=True, stop=True,
                        tile_position=(32 * s, 0),
                    )
                tq[(nt, u)] = t_ps

            def emit_unit(nt, u):
                t_ps = tq.pop((nt, u))
                e_sb = ep.tile([128, 2, NT], F8, tag="e", name=f"e{nt}_{u}")
                # exp split: ScalarE does cols [0, NT-WD), DVE does the last
                # WD via the fp8e4m3 bit-trick (u8 = round(s*EXPA + EXPB),
                # exact round-to-nearest; zero-mean so softmax stays untilted)
                nc.scalar.activation(
                    out=e_sb[:, :, 0:NT - WD], in_=t_ps[:, :, 0:NT - WD],
                    func=FT.Exp)
                if WD:
                    nc.vector.tensor_scalar(
                        out=e_sb[:, :, NT - WD:].bitcast(U8),
                        in0=t_ps[:, :, NT - WD:],
                        scalar1=EXPA, scalar2=EXPB,
                        op0=mybir.AluOpType.mult, op1=mybir.AluOpType.add)
                pending.append((nt, u, e_sb))
                while len(pending) > lag_target(nt, u):
                    pnt, pu, pe = pending.pop(0)
                    emit_oz(pnt, pu, pe)
                    if pu == NU - 1:
                        tail1(pnt)      # right after the O'/Z stop
                if u == 13 and nt + 2 <= NTILES - 1:
                    # Q for tile nt+2, off the critical path
                    qproj(nt + 2, f_psum)
                # deferred tails for the PREVIOUS tile
                if nt > 0:
                    if u == 5:
                        tail2(nt - 1, 0)
                    elif u == 6:
                        tail2(nt - 1, 1)
                    elif u == 8:
                        tail2z(nt - 1)
                    elif u == 10:
                        tail3(nt - 1, 0)
                        tail3(nt - 1, 1)

            # tile 0's units interleave with the tail of the projection chase
            # (K group g / V pairs land 2 rounds ahead of the units that use
            # them, so attention starts as soon as kstk0/qts0 are up)
            # once exp owns ScalarE, stop issuing DMA triggers from it
            dma_engs = [nc.sync, nc.gpsimd]

            sched = []
            for r in range(2, NGRP + 2):
                if r < NGRP:
                    sched.append(("proj", r))
                sched.append(("unit", (0, 2 * (r - 2))))
                sched.append(("unit", (0, 2 * (r - 2) + 1)))
            for nt in range(1, NTILES):
                for u in range(NU):
                    sched.append(("unit", (nt, u)))
            units = [a for k, a in sched if k == "unit"]
            emit_T(*units[0])
            ui = 0
            for kind, arg in sched:
                if kind == "proj":
                    r = arg
                    kproj(r)
                    vpair(2 * r, lambda: proj_psum([128, C]))
                    vpair(2 * r + 1, lambda: proj_psum([128, C]))
                else:
                    ui += 1
                    if ui < len(units):
                        emit_T(*units[ui])
                    emit_unit(*arg)
            for pnt, pu, pe in pending:
                emit_oz(pnt, pu, pe)
                if pu == NU - 1:
                    tail1(pnt)
            for cp in range(2):
                tail2(NTILES - 1, cp)
            tail2z(NTILES - 1)
            for cp in range(2):
                tail3(NTILES - 1, cp)

    _split_multiwaits(nc)
    return nc


def _split_multiwaits(nc: bass.Bass) -> None:
    """This container's walrus accepts at most ONE sync-wait per instruction
    (CoreV3GenImpl setupSyncWait). Tile emits multi-wait instructions; split
    the excess waits onto EventSemaphore carriers inserted just before the
    instruction on the same engine."""
    import json as _json

    data = _json.loads(mybir.module_to_json_bytes(nc.m))
    uid = 0
    for fn in data["functions"]:
        for bb in fn["blocks"]:
            new = []
            for inst in bb["instructions"]:
                si = inst.get("sync_info")
                waits = (si or {}).get("on_wait") or []
                if len(waits) > 1:
                    for wcmd in waits[:-1]:
                        uid += 1
                        new.append({
                            "debug": inst.get("debug", 0),
                            "engine": inst["engine"],
                            "ins": [], "outs": [],
                            "name": f"syncw-{uid}",
                            "opcode": "EventSemaphore",
                            "sync_info": {"on_update": [], "on_wait": [wcmd]},
                        })
                    si["on_wait"] = [waits[-1]]
                new.append(inst)
            bb["instructions"] = new
    nc.m = mybir.module_from_json_bytes(_json.dumps(data).encode())


_NC = None


def _get_nc():
    global _NC
    if _NC is None:
        _NC = build_nc()
    return _NC


def _prep_maps(x, Wf, bf, Wg, bg, Wh, bh, Wo, bo):
    bft = ml_dtypes.bfloat16
    shared = {
        "wf": np.ascontiguousarray(Wf.reshape(2, 128, KEY).astype(bft)),
        "wg": np.ascontiguousarray(Wg.reshape(2, 128, KEY).astype(bft)),
        "wh": np.ascontiguousarray(Wh.reshape(2, 128, C).astype(bft)),
        "wo": np.ascontiguousarray(Wo.reshape(2, 128, C).astype(bft)),
        "bfT": np.ascontiguousarray(bf.reshape(KEY, 1).astype(np.float32)),
        "bgT": np.ascontiguousarray(bg.reshape(KEY, 1).astype(np.float32)),
        "bhp": np.ascontiguousarray(bh.reshape(1, C).astype(bft)),
        "bop": np.ascontiguousarray(bo.reshape(1, C).astype(bft)),
    }
    in_maps = []
    for b in range(B):
        xTb = np.ascontiguousarray(
            x[b].reshape(N, C).T.astype(bft).reshape(2, 128, N)
        )
        m = dict(shared)
        m["xT"] = xTb
        in_maps.append(m)
    return in_maps


def run(x, Wf, bf, Wg, bg, Wh, bh, Wo, bo, trace=False, **kw):
    x = np.asarray(x, dtype=np.float32)
    in_maps = _prep_maps(
        x, *(np.asarray(a, dtype=np.float32) for a in (Wf, bf, Wg, bg, Wh, bh, Wo, bo))
    )
    res = run_bass_kernel_spmd(_get_nc(), in_maps, list(range(B)), trace=trace, **kw)
    out = np.empty((B, H, W, C), dtype=np.float32)
    for b in range(B):
        oT = np.asarray(res.results[b]["outT"], dtype=np.float32).reshape(C, N)
        out[b] = oT.T.reshape(H, W, C)
    return out, res


def kernel(x, Wf, bf, Wg, bg, Wh, bh, Wo, bo):
    out, _ = run(x, Wf, bf, Wg, bg, Wh, bh, Wo, bo)
    return out

